# revision 2
# baseline (speedup 1.0000x reference)
"""Trainium2 Bass kernel for nn_BaseAttention (B=2,S=2048,D=1024,H=16,K=64).

Sharding: 8-way tensor parallel over heads. Core c owns heads {2c, 2c+1}
and computes Q/K/V projections + attention for those heads over BOTH
batches (full S), then an AllToAll (1MB/core, bf16) redistributes the
per-head context from head-sharded to row-sharded. Each core finishes
with the full output projection + residual + LayerNorm for its 512
output rows (batch c//4, query block c%4).

Layouts (per core, hk = 2 heads x 64 = 128):
  xT  [D, 2S]   bf16 : x (both batches) transposed; contraction d on parts
  QT/KT [hk, S] bf16 : per batch, produced transposed
  va  [s128, 130] bf16 : V transposed per s-chunk with a ones column per
                  head appended -> ctx matmul also emits softmax denom
  scores PSUM [s128, 1024], exp'd on ACT in [128,1024] tiles -> et bf16
  ctx PSUM [65, 2048] per head; denom row normalized via DVE reciprocal
                  + DMA-broadcast roundtrip (PSUM is fully booked)
  AllToAll over [[0..7]]: sendb [8*128, 512] blocks; block d = my pair's
                  normalized ctx for dest d's rows
  out-proj: 8 received pair tiles [128 hk, 512 q] as stationary, wo
                  moving; residual folded in via identity matmul; LN with
                  centering folded into ACT Square bias.
"""

import sys
import numpy as np

B, S, D, H, KD = 2, 2048, 1024, 16, 64
SB = S // 4          # 512 rows per core output
HK = H * KD
P = 128
NDC = D // P         # 8 d-chunks
NSC = S // P         # 16 s-chunks per batch
NQ5 = S // 512       # 4 q-chunks of 512 per batch
EW = 1024            # exp tile width (free dim)
NEW = S // EW        # 2 exp groups per batch

if "/opt/trn_rl_repo" not in sys.path:
    sys.path.insert(0, "/opt/trn_rl_repo")

_cache = {}


def _build():
    import concourse.bass as bass
    import concourse.mybir as mybir
    from concourse.tile import TileContext

    dt = mybir.dt
    f32, f32r, bf16, f8 = dt.float32, dt.float32r, dt.bfloat16, dt.float8e4
    AF = mybir.ActivationFunctionType
    OP = mybir.AluOpType
    AX = mybir.AxisListType.X
    DR = mybir.MatmulPerfMode.DoubleRow

    nc = bass.Bass(num_devices=8)
    xT = nc.declare_dram_parameter("xT", [D, 2 * S], bf16, isOutput=False)
    wqp = nc.declare_dram_parameter("wqp", [P, D], bf16, isOutput=False)
    wkp = nc.declare_dram_parameter("wkp", [P, D], bf16, isOutput=False)
    wvp = nc.declare_dram_parameter("wvp", [P, D], bf16, isOutput=False)
    bqp = nc.declare_dram_parameter("bqp", [P, 1], f32, isOutput=False)
    bkp = nc.declare_dram_parameter("bkp", [P, 1], f32, isOutput=False)
    bvp = nc.declare_dram_parameter("bvp", [P, 1], f32, isOutput=False)
    wop = nc.declare_dram_parameter("wop", [P, 8 * D], bf16, isOutput=False)
    xqb = nc.declare_dram_parameter("xqb", [P, 4 * D], f32r, isOutput=False)
    gmp = nc.declare_dram_parameter("gamma_row", [1, D], f32, isOutput=False)
    btp = nc.declare_dram_parameter("beta_row", [1, D], f32, isOutput=False)
    idrp = nc.declare_dram_parameter("identr_in", [P, P], f32r, isOutput=False)
    idbp = nc.declare_dram_parameter("identb_in", [P, P], bf16, isOutput=False)
    out = nc.declare_dram_parameter("out", [SB, D], f32, isOutput=True)

    with TileContext(nc) as tc:
        with tc.tile_pool(name="const", bufs=1) as cpool, \
             tc.tile_pool(name="dram", bufs=2, space="DRAM") as dram, \
             tc.tile_pool(name="dramn", bufs=2, space="DRAM") as dramn, \
             tc.tile_pool(name="wq3", bufs=3) as wpool, \
             tc.tile_pool(name="qk", bufs=2) as qkp, \
             tc.tile_pool(name="vt", bufs=1) as vtp, \
             tc.tile_pool(name="va", bufs=32) as vap, \
             tc.tile_pool(name="et", bufs=12) as etp, \
             tc.tile_pool(name="cn", bufs=2) as cnp, \
             tc.tile_pool(name="rd", bufs=2) as rdp, \
             tc.tile_pool(name="rb", bufs=2) as rbp:

            identr = cpool.tile([P, P], f32r, tag="identr")
            nc.sync.dma_start(out=identr[:], in_=idrp[:])
            eps_t = cpool.tile([P, 1], f32, tag="eps")
            nc.vector.memset(eps_t[:], 1e-6)
            nsh_t = cpool.tile([P, 1], f32, tag="nsh")
            nc.vector.memset(nsh_t[:], -4.0)
            bq_sb = cpool.tile([P, 1], f32, tag="bq")
            nc.sync.dma_start(out=bq_sb[:], in_=bqp[:])
            bk_sb = cpool.tile([P, 1], f32, tag="bk")
            nc.sync.dma_start(out=bk_sb[:], in_=bkp[:])
            bv_sb = cpool.tile([P, 1], f32, tag="bv")
            nc.sync.dma_start(out=bv_sb[:], in_=bvp[:])
            wq_sb = wpool.tile([P, D], bf16, tag="wq")
            nc.sync.dma_start(out=wq_sb[:], in_=wqp[:])
            wk_sb = wpool.tile([P, D], bf16, tag="wk")
            nc.sync.dma_start(out=wk_sb[:], in_=wkp[:])
            wv_sb = wpool.tile([P, D], bf16, tag="wv")
            nc.sync.dma_start(out=wv_sb[:], in_=wvp[:])

            sendb = dram.tile([8 * P, 512], bf16)
            recvb = dram.tile([8 * P, 512], bf16)

            xt_pool = tc.tile_pool(name="xt", bufs=8)
            xtp = xt_pool.__enter__()

            # ---------- projection helpers ----------
            def load_xt(b):
                tiles = []
                for dc in range(NDC):
                    t = xtp.tile([P, S], bf16, tag="xt")
                    for half in range(2):
                        nc.sync.dma_start(
                            out=t[:, half * (S // 2):(half + 1) * (S // 2)],
                            in_=xT[dc * P:(dc + 1) * P,
                                   b * S + half * (S // 2):
                                   b * S + (half + 1) * (S // 2)])
                    tiles.append(t)
                return tiles

            def proj_chunk(qp_pool, w_sb, xt_tiles, out_tile, qs, bias):
                """One 512-wide chunk of a projection: 8 MMs + bias evict."""
                pt = qp_pool.tile([P, 512], f32, tag="qp")
                for dc in range(NDC):
                    nc.tensor.matmul(
                        pt[:], w_sb[:, dc * P:(dc + 1) * P],
                        xt_tiles[dc][:, qs * 512:(qs + 1) * 512],
                        start=(dc == 0), stop=(dc == NDC - 1))
                nc.vector.tensor_scalar(
                    out_tile[:, qs * 512:(qs + 1) * 512], pt[:], bias[:],
                    None, OP.add)

            def transpose_chunk(vt_pool, VT, va_tiles, sc):
                """Transpose VT s-chunk into fp8 DoubleRow va pair tiles.

                va_tiles[pair][h] is [128, 2, 65]: two s-chunks stacked on
                the middle dim (DoubleRow contraction pair), 64 V columns
                plus a ones column per head.
                """
                pt = vt_pool.tile([P, P], bf16, tag="vt")
                nc.tensor.transpose(
                    pt[:], VT[:, sc * P:(sc + 1) * P], identb[:])
                pair, par = sc // 2, sc % 2
                if par == 0:
                    va_tiles.append(
                        [vap.tile([P, 160], f8, tag="va",
                                  name=f"va_{len(va_tiles)}_{h}")
                         for h in range(2)])
                for h in range(2):
                    va = va_tiles[pair][h]
                    nc.vector.tensor_copy(
                        va[:, par * 80:par * 80 + 64],
                        pt[:, h * 64:(h + 1) * 64])
                    nc.vector.memset(va[:, par * 80 + 64:par * 80 + 80], 1.0)

            identb = cpool.tile([P, P], bf16, tag="identb")
            nc.sync.dma_start(out=identb[:], in_=idbp[:])

            # ---------- phase 1: batch 0 projections ----------
            with tc.tile_pool(name="qp0", bufs=2, space="PSUM") as qp0, \
                 tc.tile_pool(name="vt0", bufs=2, space="PSUM") as vt0:
                xt0 = load_xt(0)
                QT0 = qkp.tile([P, S], bf16, tag="qt")
                KT0 = qkp.tile([P, S], bf16, tag="kt")
                VT0 = vtp.tile([P, S], bf16, tag="vtmp")
                for qs in range(NQ5):
                    proj_chunk(qp0, wq_sb, xt0, QT0, qs, bq_sb)
                for qs in range(NQ5):
                    proj_chunk(qp0, wk_sb, xt0, KT0, qs, bk_sb)
                for qs in range(NQ5):
                    proj_chunk(qp0, wv_sb, xt0, VT0, qs, bv_sb)
                va0 = []
                for sc in range(NSC):
                    transpose_chunk(vt0, VT0, va0, sc)

            # ---------- phases 2-3: attention (b) + interleaved work ----
            LAGP = 4  # ctx pair-matmuls trail scores by LAGP s-chunk
            # pairs: when the first ctx of a q-half waits on pc (freed by
            # the previous half's normalize), the in-order PE stream must
            # not stall the scores queued behind it.
            NPAIR = NSC // 2

            def attn_unit(b, hl, QT, KT, va, cn, filler):
                """Attention for local head hl of one batch, in fp8 with
                DoubleRow (contraction pairs on the middle AP dim).

                q processed in two 1024-halves so pc fits 2 PSUM banks and
                sps can double-buffer. filler: list of zero-arg closures
                emitting ~1 matmul of interleavable PE work each; drained
                into exp-wait gaps.
                """
                for qh in range(NEW):
                    pc = cps.tile([80, EW], f32, tag="pc")

                    def emit_ctx(pair, et3):
                        va3 = va[pair][hl][:].rearrange(
                            "p (g c) -> p g c", g=2)
                        for qcl in range(EW // 512):
                            nc.tensor.matmul(
                                pc[0:80, qcl * 512:(qcl + 1) * 512],
                                va3,
                                et3[:, :, qcl * 512:(qcl + 1) * 512],
                                start=(pair == 0), stop=(pair == NPAIR - 1),
                                perf_mode=DR)

                    pend = []
                    for sc in range(NSC):
                        par = sc % 2
                        if par == 0:
                            et8 = etp.tile([P, 2 * EW], f8, tag="et")
                        sps = spsp.tile([P, EW], f32, tag="sps")
                        for qcl in range(EW // 512):
                            nc.tensor.matmul(
                                sps[:, qcl * 512:(qcl + 1) * 512],
                                KT[hl * 64:(hl + 1) * 64, sc * P:(sc + 1) * P],
                                QT[hl * 64:(hl + 1) * 64,
                                   qh * EW + qcl * 512:
                                   qh * EW + (qcl + 1) * 512],
                                start=True, stop=True)
                        # bias -4: keeps exp within fp8e4m3 range (max
                        # |score| ~6.5); numerator and ones-column
                        # denominator scale identically so probs are exact
                        nc.scalar.activation(
                            et8[:, par * EW:(par + 1) * EW], sps[:], AF.Exp,
                            bias=nsh_t[:])
                        if par == 1:
                            pend.append(
                                (sc // 2,
                                 et8[:].rearrange("p (g q) -> p g q", g=2)))
                            if len(pend) > LAGP:
                                emit_ctx(*pend.pop(0))
                        if filler and (sc % 2 == 1):
                            filler.pop(0)()
                    for it in pend:
                        emit_ctx(*it)
                    # normalize this q-half: reciprocal the denom row, DMA
                    # broadcast it to 64 partitions, multiply
                    rd = rdp.tile([65, EW], f32, tag="rd")
                    nc.vector.reciprocal(rd[64:65, :], pc[64:65, :])
                    dn = dramn.tile([1, EW], f32)
                    nc.sync.dma_start(out=dn[:], in_=rd[64:65, :])
                    rb = rbp.tile([64, EW], f32, tag="rb")
                    for half in range(2):
                        nc.sync.dma_start(
                            out=rb[:, half * (EW // 2):(half + 1) * (EW // 2)],
                            in_=dn[:, half * (EW // 2):(half + 1) * (EW // 2)]
                            .to_broadcast((64, EW // 2)))
                    nc.vector.tensor_tensor(
                        cn[hl * 64:(hl + 1) * 64, qh * EW:(qh + 1) * EW],
                        pc[0:64, :], rb[:], OP.mult)
                    if hl == 1:
                        # both heads of this q-half done: ship its 2 blocks
                        for qb in (2 * qh, 2 * qh + 1):
                            nc.sync.dma_start(
                                out=sendb[(b * 4 + qb) * P:
                                          (b * 4 + qb + 1) * P, :],
                                in_=cn[:, qb * 512:(qb + 1) * 512])

            with tc.tile_pool(name="sps", bufs=2, space="PSUM") as spsp, \
                 tc.tile_pool(name="cps", bufs=1, space="PSUM") as cps, \
                 tc.tile_pool(name="qp1", bufs=1, space="PSUM") as qp1, \
                 tc.tile_pool(name="vt1", bufs=1, space="PSUM") as vt1:

                # b1 projection work, chopped into filler closures
                xt1 = load_xt(1)
                QT1 = qkp.tile([P, S], bf16, tag="qt")
                KT1 = qkp.tile([P, S], bf16, tag="kt")
                VT1 = vtp.tile([P, S], bf16, tag="vtmp")
                va1 = []
                cn0 = cnp.tile([P, S], bf16, tag="cn")
                cn1 = cnp.tile([P, S], bf16, tag="cn")

                filler = []
                for qs in range(NQ5):
                    filler.append(lambda qs=qs: proj_chunk(
                        qp1, wq_sb, xt1, QT1, qs, bq_sb))
                for qs in range(NQ5):
                    filler.append(lambda qs=qs: proj_chunk(
                        qp1, wk_sb, xt1, KT1, qs, bk_sb))
                for qs in range(NQ5):
                    filler.append(lambda qs=qs: proj_chunk(
                        qp1, wv_sb, xt1, VT1, qs, bv_sb))
                for sc in range(NSC):
                    filler.append(lambda sc=sc: transpose_chunk(
                        vt1, VT1, va1, sc))

                attn_unit(0, 0, QT0, KT0, va0, cn0, filler)
                attn_unit(0, 1, QT0, KT0, va0, cn0, filler)

                # drain any leftover b1 prep before b1 attention needs it
                while filler:
                    filler.pop(0)()

                attn_unit(1, 0, QT1, KT1, va1, cn1, None)
                attn_unit(1, 1, QT1, KT1, va1, cn1, None)

            xt_pool.__exit__(None, None, None)

            # ---------- exchange ----------
            nc.gpsimd.collective_compute(
                "AllToAll", mybir.AluOpType.bypass,
                replica_groups=[[0, 1, 2, 3, 4, 5, 6, 7]],
                ins=[sendb[:]], outs=[recvb[:]])

            # ---------- out-projection + residual + LayerNorm ----------
            with tc.tile_pool(name="wo", bufs=1) as wopool, \
                 tc.tile_pool(name="lnB", bufs=1) as lbp, \
                 tc.tile_pool(name="ctxa", bufs=8) as ctxp, \
                 tc.tile_pool(name="ln", bufs=2) as lnp, \
                 tc.tile_pool(name="st", bufs=8) as stp, \
                 tc.tile_pool(name="ops", bufs=2, space="PSUM") as ops:
                wo_sb = wopool.tile([P, 8 * D], bf16, tag="wo")
                for j in range(8):
                    nc.sync.dma_start(out=wo_sb[:, j * D:(j + 1) * D],
                                      in_=wop[:, j * D:(j + 1) * D])
                xq_sb = wopool.tile([P, 4 * D], f32r, tag="xqb")
                nc.sync.dma_start(out=xq_sb[:], in_=xqb[:])
                gmB = lbp.tile([P, D], f32, tag="gmB")
                btB = lbp.tile([P, D], f32, tag="btB")
                nc.sync.dma_start(out=gmB[:], in_=gmp[:].to_broadcast((P, D)))
                nc.sync.dma_start(out=btB[:], in_=btp[:].to_broadcast((P, D)))
                ctx_all = []
                for j in range(8):
                    t = ctxp.tile([P, 512], bf16, tag="ctxa")
                    nc.sync.dma_start(out=t[:], in_=recvb[j * P:(j + 1) * P, :])
                    ctx_all.append(t)

                # keep the PE clock ramped while waiting on the collective:
                # a stream of dependency-free matmuls into a scratch bank
                warm = ops.tile([P, 512], f32, tag="warm")
                for _ in range(96):
                    nc.tensor.matmul(warm[:], identb[:], cn1[:, 0:512],
                                     start=True, stop=True)

                for qc in range(4):
                    po = ops.tile([P, D], f32, tag="ops")
                    for j in range(8):
                        for d5 in range(2):
                            nc.tensor.matmul(
                                po[:, d5 * 512:(d5 + 1) * 512],
                                ctx_all[j][:, qc * P:(qc + 1) * P],
                                wo_sb[:, j * D + d5 * 512:
                                      j * D + (d5 + 1) * 512],
                                start=(j == 0), stop=False)
                    for d5 in range(2):
                        nc.tensor.matmul(
                            po[:, d5 * 512:(d5 + 1) * 512],
                            identr[:],
                            xq_sb[:, qc * D + d5 * 512:qc * D + (d5 + 1) * 512],
                            start=False, stop=True)
                    yt = lnp.tile([P, D], f32, tag="yt")
                    sum_t = stp.tile([P, 1], f32, tag="sum")
                    nc.scalar.activation(yt[:], po[:], AF.Copy,
                                         accum_out=sum_t[:])
                    nmean = stp.tile([P, 1], f32, tag="nmean")
                    nc.vector.tensor_scalar_mul(nmean[:], sum_t[:], -1.0 / D)
                    sq = lnp.tile([P, D], f32, tag="sq")
                    vs = stp.tile([P, 1], f32, tag="vs")
                    nc.scalar.activation(sq[:], yt[:], AF.Square,
                                         bias=nmean[:], scale=1.0,
                                         accum_out=vs[:])
                    std = stp.tile([P, 1], f32, tag="std")
                    nc.scalar.activation(std[:], vs[:], AF.Sqrt,
                                         bias=eps_t[:], scale=1.0 / D)
                    rstd = stp.tile([P, 1], f32, tag="rstd")
                    nc.vector.reciprocal(rstd[:], std[:])
                    t1 = lnp.tile([P, D], f32, tag="t1")
                    nc.vector.tensor_scalar(t1[:], yt[:], nmean[:], rstd[:],
                                            OP.add, OP.mult)
                    nc.vector.tensor_tensor(t1[:], t1[:], gmB[:], OP.mult)
                    nc.vector.tensor_tensor(t1[:], t1[:], btB[:], OP.add)
                    nc.sync.dma_start(out=out[qc * P:(qc + 1) * P, :],
                                      in_=t1[:])

    # Post-pass: walrus's per-instruction ISA structs hold only ONE sync
    # wait for compute-engine instructions. Move excess waits onto
    # standalone EventSemaphore instructions placed just before on the same
    # engine stream (sequencer executes them in order; semantics unchanged).
    SPLIT = {"InstMatmult", "InstTensorScalarPtr", "InstTensorScalar",
             "InstTensorTensor", "InstReciprocal", "InstActivation",
             "InstTensorReduce", "InstTensorCopy", "InstMemSet",
             "InstCopy", "InstDMACopy", "InstDMATranspose", "InstDrain",
             "InstCollectiveCompute", "InstLdweights"}
    evt_n = 0
    for f in nc.m.functions:
        for bb in f.blocks:
            need = any(
                type(i).__name__ in SPLIT and i.sync_info is not None
                and len(i.sync_info.on_wait) > 1 for i in bb.instructions)
            if not need:
                continue
            newl = []
            for ins in bb.instructions:
                si = ins.sync_info
                if (type(ins).__name__ in SPLIT and si is not None
                        and len(si.on_wait) > 1):
                    extra = list(si.on_wait[:-1])
                    for j in range(0, len(extra), 2):  # evt-sem holds <=2
                        evt_n += 1
                        evt = mybir.InstEventSemaphore(name=f"mmwait_{evt_n}")
                        evt.engine = ins.engine
                        evt.sync_info = mybir.SyncInfo(
                            on_wait=extra[j:j + 2], on_update=[])
                        newl.append(evt)
                    ins.sync_info = mybir.SyncInfo(
                        on_wait=[si.on_wait[-1]],
                        on_update=list(si.on_update))
                newl.append(ins)
            bb.instructions = newl
    return nc


def get_nc():
    if "nc" not in _cache:
        _cache["nc"] = _build()
    return _cache["nc"]


def make_in_maps(inputs, n_cores=8):
    """Shard full inputs into per-core input maps (host-side layout prep)."""
    import ml_dtypes
    bf = ml_dtypes.bfloat16
    f = np.float32
    x = np.asarray(inputs["x"], f)                       # [B, S, D]
    wq = np.asarray(inputs["wq"], f).reshape(D, HK)
    wk = np.asarray(inputs["wk"], f).reshape(D, HK)
    wv = np.asarray(inputs["wv"], f).reshape(D, HK)
    wo = np.asarray(inputs["wo"], f).reshape(HK, D)
    bq = np.asarray(inputs["bq"], f).reshape(HK)
    bk = np.asarray(inputs["bk"], f).reshape(HK)
    bv = np.asarray(inputs["bv"], f).reshape(HK)
    bo = np.asarray(inputs["bo"], f).reshape(D)
    gm = np.asarray(inputs["gamma"], f).reshape(1, D)
    bt = np.asarray(inputs["beta"], f).reshape(1, D)

    scale = 1.0 / np.sqrt(KD)
    # xT for both batches: [D, 2S]
    xTb = np.ascontiguousarray(
        np.concatenate([x[0].T, x[1].T], axis=1)).astype(bf)
    # wo packed: wop[p, j*D + col] = wo[j*128 + p, col]
    wop = np.ascontiguousarray(
        wo.reshape(8, P, D).transpose(1, 0, 2).reshape(P, 8 * D)).astype(bf)

    def pack_w(w):  # [D, 128 my hk] -> [128, D] d-chunk-major cols
        return np.ascontiguousarray(
            w.reshape(NDC, P, P).transpose(1, 0, 2).reshape(P, D))

    maps = []
    for c in range(n_cores):
        h0 = 2 * c * KD
        wq_my = pack_w(wq[:, h0:h0 + P] * scale).astype(bf)
        wk_my = pack_w(wk[:, h0:h0 + P]).astype(bf)
        wv_my = pack_w(wv[:, h0:h0 + P]).astype(bf)
        b_own, qb_own = c // 4, c % 4
        xrows = x[b_own, qb_own * SB:(qb_own + 1) * SB] + bo  # [512, D]
        xqb_my = np.ascontiguousarray(
            xrows.reshape(4, P, D).transpose(1, 0, 2).reshape(P, 4 * D))
        maps.append(dict(
            xT=xTb, wqp=wq_my, wkp=wk_my, wvp=wv_my,
            bqp=(bq[h0:h0 + P] * scale).reshape(P, 1),
            bkp=bk[h0:h0 + P].reshape(P, 1),
            bvp=bv[h0:h0 + P].reshape(P, 1),
            wop=wop, xqb=xqb_my, gamma_row=gm, beta_row=bt,
            identr_in=np.eye(P, dtype=f), identb_in=np.eye(P).astype(bf),
        ))
    return maps


def kernel(**inputs):
    from concourse.bass_utils import run_bass_kernel_spmd
    nc = get_nc()
    maps = make_in_maps(inputs)
    res = run_bass_kernel_spmd(nc, maps, list(range(8)))
    outp = np.empty((B, S, D), np.float32)
    for c in range(8):
        b, qb = c // 4, c % 4
        outp[b, qb * SB:(qb + 1) * SB] = res.results[c]["out"]
    return outp
